# revision 48
# baseline (speedup 1.0000x reference)
"""Trainium2 Bass kernel for quantized-linear + LoRA (nn_LoRALinear).

Computes, for x:(4,2048,4096) f32, weight_quant:(4096,4096) i32 in [0,16),
scale/zero:(4096,1) f32, lora_A:(16,4096), lora_B:(4096,16), bias:(4096,):

    W = (weight_quant - zero) * scale
    y = x @ W.T + bias + 2.0 * (x @ lora_A.T) @ lora_B.T

Sharding across 8 NeuronCores: 4-way over tokens x 2-way over out-features.
Per core: x-slice (2048, 4096), weight rows slice (2048 of 4096), output
block (2048 tokens, 2048 features); host slices/permutes/dtype-repacks
inputs into partition-major blocks (so every DMA moves 4-16KB contiguous
per partition) and stitches/transposes output blocks.

Device algorithm (per core):

    P[o,n]   = sum_d (wq[o,d]-8) * x8[d,n]         (PE; all 32 d-chunks as
                                                    fp8e4 DoubleRow pairs)
             + sum_r B2[o,r] * t[r,n]              (K=17 fp32r matmul into the
             + (8-zero[o]) * rowsum[n]              same psum accumulation,
                                                    4x row-tiled)
    y[o,n]   = scale[o] * P[o,n] + bias[o]         (psum eviction)

with t = lora_A @ x.T (bf16, 4x column-tiled over d-chunks) augmented by a
ones-row giving rowsum, B2 = 2*lora_B/scale. The host ships x in bf16 and
fp8e4 (x8) and the centered weights (wq-8) in fp8e4 (exact for ints in
[-8,7]). The four column-tiled partial t groups are merged and replicated
across row-groups by one matmul against a host-built 0/1 matrix (DVE lanes
cannot cross partitions; the PE can). Output stays transposed per o-tile
group; the host de-transposes.
"""
import os
import sys
import types

sys.path.insert(0, "/opt/trn_rl_repo")

import numpy as np

import concourse.bass as bass
import concourse.mybir as mybir
import concourse.tile as tile
from concourse import bacc
from concourse.bass_utils import run_bass_kernel_spmd
from concourse.masks import make_identity

F32 = mybir.dt.float32
F32R = mybir.dt.float32r
BF16 = mybir.dt.bfloat16
FP8 = mybir.dt.float8e4
U8 = mybir.dt.uint8

DR = mybir.MatmulPerfMode.DoubleRow

NP_BF16 = mybir.dt.np(BF16)
NP_FP8 = mybir.dt.np(FP8)

# Problem shape (hardcoded per contract)
B, S, D, O, R = 4, 2048, 4096, 4096, 16
SCALING = 32.0 / 16.0
N_TOK = B * S            # 8192 tokens
T_SH, F_SH = 4, 2        # token shards x feature shards = 8 cores
N_SH = N_TOK // T_SH     # 2048 tokens per core
O_SH = O // F_SH         # 2048 out-features per core

NT = 4                   # n tiles per core
N_TILE = N_SH // NT      # 512
KC = D // 128            # 32 contraction chunks of 128
KQ = 4                   # k quarters (8 chunks each)
KD = KC // 2             # 16 DoubleRow chunk-pairs
OT = O_SH // 128         # 16 o tiles
OQ = 4                   # o tiles per psum pass (= o-column groups)
WQ_CENTER = 8.0          # weights shipped centered: wq - 8 (exact in fp8e4)


def _ensure_ntff_hook():
    """Best-effort: register the axon NTFF profile hook so trace=True works."""
    try:
        import antenv
        if "antenv.axon_hooks" not in sys.modules:
            hooks_mod = types.ModuleType("antenv.axon_hooks")
            hooks_mod._hook = None
            hooks_mod.set_axon_ntff_profile_hook = lambda h: setattr(hooks_mod, "_hook", h)
            hooks_mod.get_axon_ntff_profile_hook = lambda: hooks_mod._hook
            sys.modules["antenv.axon_hooks"] = hooks_mod
            antenv.axon_hooks = hooks_mod
        from trn_agent_boot.trn_boot import _ntff_profile_via_ctypes
        sys.modules["antenv.axon_hooks"].set_axon_ntff_profile_hook(
            _ntff_profile_via_ctypes("/opt/axon/libaxon_pjrt.so")
        )
        import concourse.bass_utils as bu
        bu.upload_artifacts = lambda tmpdir: tmpdir
    except Exception:
        pass


def build_nc() -> bass.Bass:
    nc = bacc.Bacc("TRN2", target_bir_lowering=False, debug=False)

    # host-pre-arranged partition-major layouts (big DMA packets):
    #  xt/xt8 [NT, KQ, 128, 8, N_TILE]   x.T blocks, bf16 / fp8
    #  wqt    [OQ, 128, KC, 512]         wq-8 per o-column group, fp8
    #  sbz    [128, 48]                  scale | bias | zero as [p, ot]
    #  y      [NT, OQ, 128, OQ, N_TILE]  transposed output blocks
    # x ships as two fp8 planes: x8 = fp8(bf16(x)) for the DR mains, and
    # xr8 = fp8(bf16(x) - x8), so lora/rowsum see x8+xr8 ~= bf16(x) while
    # moving half the bytes of a bf16 plane.
    xt_d = nc.dram_tensor("xt", (NT, KQ, 128, 8, N_TILE), FP8, kind="ExternalInput")
    xt8_d = nc.dram_tensor("xt8", (NT, KQ, 128, 8, N_TILE), FP8, kind="ExternalInput")
    wqt_d = nc.dram_tensor("wqt", (OQ, 128, KC, 512), FP8, kind="ExternalInput")
    sbz_d = nc.dram_tensor("sbz", (128, 3 * OT), F32, kind="ExternalInput")
    a_d = nc.dram_tensor("lora_at", (128, KC, R), BF16, kind="ExternalInput")
    b_d = nc.dram_tensor("lora_b", (128, OT * R), F32, kind="ExternalInput")
    # merge+replicate matrix: out[32j+r] = sum_i in[32i+r] for r<=16
    repm_d = nc.dram_tensor("repm", (128, 128), F32, kind="ExternalInput")
    y_d = nc.dram_tensor("y", (NT, OQ, 128, OQ, N_TILE), F32, kind="ExternalOutput")

    with tile.TileContext(nc) as tc:
        with (
            tc.tile_pool(name="const", bufs=1) as cpool,
            tc.tile_pool(name="wt", bufs=1) as wtpool,
            tc.tile_pool(name="xt", bufs=2) as xtpool,
            tc.tile_pool(name="xt8", bufs=2) as xt8pool,
            tc.tile_pool(name="stage", bufs=2) as stage,
            tc.tile_pool(name="tsb", bufs=2) as tsbpool,
            tc.tile_pool(name="outp", bufs=4) as outp,
            tc.tile_pool(name="ps_acc", bufs=7, space="PSUM") as ps_accp,
            tc.tile_pool(name="ps_t", bufs=1, space="PSUM") as ps_tp,
        ):
            # ---------------- PE warm-up: dependency-free matmuls so the HAM
            # clock-gate reaches K=8/8 before the real mains arrive.
            zeros_w = cpool.tile([128, N_TILE], BF16)
            nc.gpsimd.memset(zeros_w[:], 0.0)
            for _ in range(16):
                wps = ps_accp.tile([128, N_TILE], F32, tag="acc", name="wps")
                nc.tensor.matmul(
                    wps[:], zeros_w[:, 0:128], zeros_w[:],
                    start=True, stop=True,
                )

            # ---------------- constants (emitted onto the scalar queue
            # after nt0's first x-resid quarters; b2 needs them only by ~20us)
            sbz_sb = cpool.tile([128, 3 * OT], F32)
            scale_sb = sbz_sb[:, 0:OT]
            bias_sb = sbz_sb[:, OT:2 * OT]
            zero_sb = sbz_sb[:, 2 * OT:3 * OT]
            b_sb = cpool.tile([128, OT * R], F32)
            bblks = [b_sb[:, ot * R:(ot + 1) * R] for ot in range(OT)]
            repm_f = cpool.tile([128, 128], F32)

            def emit_const_loads():
                nc.scalar.dma_start(sbz_sb[:], sbz_d[:, :])
                nc.scalar.dma_start(b_sb[:], b_d[:, :])
                nc.scalar.dma_start(repm_f[:], repm_d[:, :])
                nc.vector.reciprocal(rcp_sb[:], scale_sb)
                nc.vector.tensor_scalar_mul(rcp2_sb[:], rcp_sb[:], float(SCALING))
                nc.vector.tensor_copy(repm[:], repm_f[:])

            ident_f = cpool.tile([128, 128], F32)
            make_identity(nc, ident_f)
            ident_r = cpool.tile([128, 128], F32R)
            nc.vector.tensor_copy(ident_r[:], ident_f[:])
            rcp_sb = cpool.tile([128, OT], F32)
            rcp2_sb = cpool.tile([128, OT], F32)
            repm = cpool.tile([128, 128], F32R)

            # A_augT [128, KC, 17] bf16: cols 0..15 = A.T chunk, col16 = ones.
            a4 = cpool.tile([128, KC, R + 1], BF16)
            nc.gpsimd.memset(a4[:, :, R:R + 1], 1.0)
            a_stg = cpool.tile([128, KC, R], BF16)

            def emit_a_load():
                nc.gpsimd.dma_start(a_stg[:], a_d[:, :, :])
                nc.vector.tensor_copy(a4[:, :, 0:R], a_stg[:])

            # b2r4 [128, OQ, 128] fp32r: for o-tile ot = oq*4+j, partition rows
            # 32j..32j+15 = (2*B/scale).T, row 32j+16 = (8 - zero) [pairs with
            # the rowsum row of t4]. Bias is folded into the psum eviction.
            # Built per o-group via one zero-padded [128,128] transpose so the
            # rows land at partition offsets 32j directly. Emitted after
            # mains(0,0) so it never blocks the head of the PE queue.
            b2r4 = cpool.tile([128, OQ, 128], F32R)

            def emit_b2_build():
                for oq in range(OQ):
                    pre32 = stage.tile([128, 128], F32R, tag="pre32")
                    nc.vector.tensor_copy(pre32[:], zeros_w[:, 0:128])
                    for j in range(4):
                        ot = oq * 4 + j
                        c = 32 * j
                        nc.vector.tensor_scalar(
                            out=pre32[:, c:c + R], in0=bblks[ot],
                            scalar1=rcp2_sb[:, ot:ot + 1], scalar2=None,
                            op0=mybir.AluOpType.mult,
                        )
                        nc.vector.tensor_scalar(
                            out=pre32[:, c + R:c + R + 1],
                            in0=zero_sb[:, ot:ot + 1],
                            scalar1=-1.0, scalar2=float(WQ_CENTER),
                            op0=mybir.AluOpType.mult, op1=mybir.AluOpType.add,
                        )
                    psb = ps_accp.tile([128, 128], F32R, tag="acc", name="psb")
                    nc.tensor.transpose(psb[:], pre32[:], ident_r[:])
                    nc.vector.tensor_copy(b2r4[:, oq, :], psb[:].bitcast(F32))

            # ---------------- weights: resident fp8, one tile per o-col group
            wt8s = [
                wtpool.tile([128, KC, 512], FP8, tag=f"wog{og}", name="wt8g")
                for og in range(OQ)
            ]

            def load_og(og, nsplit=1):
                kstep = KC // nsplit
                for s in range(nsplit):
                    k0, k1 = s * kstep, (s + 1) * kstep
                    nc.sync.dma_start(
                        wt8s[og][:, k0:k1, :], wqt_d[og, :, k0:k1, :]
                    )

            # ---------------- x loads (bf16 on scalar q, fp8 on gpsimd q) ----
            def alloc_x():
                xT = xtpool.tile([128, KC, N_TILE], FP8, tag="xT")
                xT8 = xt8pool.tile([128, KC, N_TILE], FP8, tag="xT8")
                return xT, xT8

            def load_x_quarter(nt, q, xT):
                nc.scalar.dma_start(
                    xT[:, 8 * q:8 * (q + 1), :], xt_d[nt, q, :, :, :]
                )

            def load_x8_quarter(nt, q, xT8, eng=None):
                (eng or nc.gpsimd).dma_start(
                    xT8[:, 8 * q:8 * (q + 1), :], xt8_d[nt, q, :, :, :]
                )

            def load_x_quarter2(nt, q, xT, eng):
                eng.dma_start(
                    xT[:, 8 * q:8 * (q + 1), :], xt_d[nt, q, :, :, :]
                )

            # ---------------- per-n-tile pieces ----------------
            def lora_proj(nt, xT, xT8):
                # t_aug: rows 0..15 = A@x.T, row16 = rowsum; 4x column-tiled
                # over d-chunks (chunk k -> col group k%4, output partitions
                # 32j..32j+16), then merged+replicated via repm on the PE.
                # Two accumulation passes: the x8 plane plus the residual.
                ps_t = ps_tp.tile([128, N_TILE], F32, tag="ps_t")
                for pi, xsrc in enumerate((xT8, xT)):
                    for t in range(KC // 4):
                        for j in range(4):
                            k = 4 * t + j
                            nc.tensor.matmul(
                                ps_t[32 * j:32 * j + R + 1, :],
                                a4[:, k, :], xsrc[:, k, :],
                                start=(pi == 0 and t == 0),
                                stop=(pi == 1 and t == KC // 4 - 1),
                                tile_position=(0, 32 * j),
                            )
                ts_sb = tsbpool.tile([128, N_TILE], F32R, tag="ts")
                if nt < 2:
                    # first use of each rotating buffer: zero it so the merge
                    # matmul sees no garbage in the never-written rows
                    nc.vector.tensor_copy(ts_sb[:], zeros_w[:])
                for j in range(4):
                    nc.vector.tensor_copy(
                        ts_sb[32 * j:32 * j + R + 1, :],
                        ps_t[32 * j:32 * j + R + 1, :],
                    )
                ps_t2 = ps_tp.tile([128, N_TILE], F32, tag="ps_t")
                nc.tensor.matmul(
                    ps_t2[:], repm[:], ts_sb[:],
                    start=True, stop=True,
                )
                t4 = tsbpool.tile([128, N_TILE], F32R, tag="t4")
                nc.vector.tensor_copy(t4[:], ps_t2[:])
                return t4

            def mains(oq, xT8):
                accs = []
                for _oi in range(OQ):
                    acc_tile = ps_accp.tile([128, N_TILE], F32, tag="acc")
                    accs.append(acc_tile)
                for kd in range(KD):
                    for oi in range(OQ):
                        nc.tensor.matmul(
                            accs[oi][:],
                            wt8s[oq][:, 2 * kd:2 * kd + 2,
                                     oi * 128:(oi + 1) * 128],
                            xT8[:, 2 * kd:2 * kd + 2, :],
                            start=(kd == 0), stop=False,
                            perf_mode=DR,
                        )
                return accs

            def tail(nt, oq, accs, t4):
                # lora + zero-correction: 4 concurrent row-tiled K=17 fp32r
                # matmuls closing the 4 accumulation banks.
                for j in range(OQ):
                    nc.tensor.matmul(
                        accs[j][:],
                        b2r4[32 * j:32 * j + R + 1, oq, :],
                        t4[32 * j:32 * j + R + 1, :],
                        start=False, stop=True,
                        tile_position=(32 * j, 0),
                    )
                yT4 = outp.tile([128, OQ, N_TILE], F32, tag="yT")
                for j in range(OQ):
                    ot = oq * OQ + j
                    # psum eviction y = scale[o]*P + bias[o]; last n-tile
                    # splits across engines so the final stores start sooner
                    if nt == NT - 1 and j % 2 == 1:
                        nc.vector.tensor_scalar(
                            out=yT4[:, j, :], in0=accs[j][:],
                            scalar1=scale_sb[:, ot:ot + 1],
                            scalar2=bias_sb[:, ot:ot + 1],
                            op0=mybir.AluOpType.mult,
                            op1=mybir.AluOpType.add,
                        )
                    else:
                        nc.scalar.activation(
                            yT4[:, j, :], accs[j][:],
                            mybir.ActivationFunctionType.Identity,
                            scale=scale_sb[:, ot:ot + 1],
                            bias=bias_sb[:, ot:ot + 1],
                        )
                # one batched store per o-group (8KB per partition); spread
                # late stores across both queues to cut the end tail.
                if nt >= NT - 2:
                    nc.sync.dma_start(y_d[nt, oq, :, 0:1, :], yT4[:, 0:1, :])
                    nc.scalar.dma_start(y_d[nt, oq, :, 1:2, :], yT4[:, 1:2, :])
                    nc.gpsimd.dma_start(y_d[nt, oq, :, 2:3, :], yT4[:, 2:3, :])
                    nc.sync.dma_start(y_d[nt, oq, :, 3:4, :], yT4[:, 3:4, :])
                else:
                    nc.sync.dma_start(y_d[nt, oq, :, :, :], yT4[:])

            # ---------------- schedule ----------------
            # startup: interleave og0 k-quarters with x8 halves on the fast
            # sync queue; bf16-x split between scalar and gpsimd queues.
            xT0, xT8_0 = alloc_x()
            # sync: og0-k0, x8-q0, rest of og0; gpsimd: x8-q1..q3 + a in
            # parallel -- the first mains need og0-k0 AND x8-q0 immediately.
            nc.sync.dma_start(wt8s[0][:, 0:8, :], wqt_d[0, :, 0:8, :])
            load_x8_quarter(0, 0, xT8_0, eng=nc.sync)
            load_x8_quarter(0, 1, xT8_0)
            nc.sync.dma_start(wt8s[0][:, 8:16, :], wqt_d[0, :, 8:16, :])
            load_x8_quarter(0, 2, xT8_0, eng=nc.sync)
            load_x8_quarter(0, 3, xT8_0)
            emit_a_load()
            nc.sync.dma_start(wt8s[0][:, 16:32, :], wqt_d[0, :, 16:32, :])
            load_x_quarter2(0, 0, xT0, nc.scalar)
            load_x_quarter2(0, 1, xT0, nc.gpsimd)
            emit_const_loads()
            load_x_quarter2(0, 2, xT0, nc.scalar)
            load_x_quarter2(0, 3, xT0, nc.gpsimd)

            xs = {0: (xT0, xT8_0)}
            for nt in range(NT):
                xT, xT8 = xs[nt]
                for oq in range(OQ):
                    accs = mains(oq, xT8)
                    if nt == 0 and oq == 0:
                        load_og(1)
                        emit_b2_build()
                    if oq == 0:
                        t4 = lora_proj(nt, xT, xT8)
                    tail(nt, oq, accs, t4)
                    if nt == 0 and oq == 0:
                        load_og(2)
                    if nt == 0 and oq == 1:
                        load_og(3)
                    # prefetch next n-tile's x, one quarter per oq pass
                    # (x8 on the fast sync queue; bf16 alternates scalar/gpsimd)
                    if nt + 1 < NT:
                        if oq == 0:
                            xs[nt + 1] = alloc_x()
                        nxT, nxT8 = xs[nt + 1]
                        load_x8_quarter(nt + 1, oq, nxT8, eng=nc.sync)
                        load_x_quarter2(nt + 1, oq, nxT,
                                        nc.scalar if oq % 2 == 0 else nc.gpsimd)

    nc.finalize()
    return nc


_NC_CACHE: dict = {}


def _get_nc() -> bass.Bass:
    if "nc" not in _NC_CACHE:
        _ensure_ntff_hook()
        _NC_CACHE["nc"] = build_nc()
    return _NC_CACHE["nc"]


def kernel(x, weight_quant, scale, zero, lora_A, lora_B, bias):
    x = np.ascontiguousarray(np.asarray(x, dtype=np.float32)).reshape(N_TOK, D)
    weight_quant = np.asarray(weight_quant, dtype=np.int32)
    scale_f = np.asarray(scale, dtype=np.float32).reshape(O)
    zero_f = np.asarray(zero, dtype=np.float32).reshape(O)
    bias_f = np.asarray(bias, dtype=np.float32).reshape(O)
    lora_A = np.asarray(lora_A, dtype=np.float32)
    lora_B = np.ascontiguousarray(np.asarray(lora_B, dtype=np.float32))

    nc = _get_nc()

    # host-side layout marshaling (slices/transposes/dtype repacks only)
    # [r, d] -> [128, KC, R]: a_t[p, k, r] = A[r, k*128 + p]
    lora_at = np.ascontiguousarray(
        lora_A.T.astype(NP_BF16).reshape(KC, 128, R).transpose(1, 0, 2))
    xt_by_t = []
    xt8_by_t = []
    for ti in range(T_SH):
        xsh = x[ti * N_SH:(ti + 1) * N_SH].astype(NP_BF16)          # [n, d]
        # [d, n] -> [q, kk, p, nt, n] -> [nt, q, p, kk, n]
        xb = xsh.T.reshape(KQ, 8, 128, NT, N_TILE).transpose(3, 0, 2, 1, 4)
        x8 = xb.astype(NP_FP8)
        xr = (xb.astype(np.float32)
              - x8.astype(np.float32)).astype(NP_FP8)                # residual
        xt_by_t.append(np.ascontiguousarray(xr))
        xt8_by_t.append(np.ascontiguousarray(x8))
    w8 = (weight_quant - 8).astype(np.float32).astype(NP_FP8)        # exact
    wqt_by_f = []
    for fi in range(F_SH):
        w8T = w8[fi * O_SH:(fi + 1) * O_SH].T                        # [d, o]
        # [d, o] -> [k, p, og, o] -> [og, p, k, o]
        wqt_by_f.append(np.ascontiguousarray(
            w8T.reshape(KC, 128, OQ, 512).transpose(2, 1, 0, 3)
        ))
    repm = np.zeros((128, 128), dtype=np.float32)
    for i in range(4):
        for j in range(4):
            for r in range(R + 1):
                repm[32 * i + r, 32 * j + r] = 1.0

    in_maps = []
    for core in range(T_SH * F_SH):
        ti, fi = core % T_SH, core // T_SH
        osl = slice(fi * O_SH, (fi + 1) * O_SH)
        sbz = np.concatenate([
            scale_f[osl].reshape(OT, 128).T,
            bias_f[osl].reshape(OT, 128).T,
            zero_f[osl].reshape(OT, 128).T,
        ], axis=1)                                                   # [128, 48]
        in_maps.append({
            "xt": xt_by_t[ti],
            "xt8": xt8_by_t[ti],
            "wqt": wqt_by_f[fi],
            "sbz": np.ascontiguousarray(sbz),
            "lora_at": lora_at,
            "lora_b": np.ascontiguousarray(
                lora_B[osl].reshape(OT, 128, R).transpose(1, 0, 2)
                .reshape(128, OT * R)),
            "repm": repm,
        })

    trace = bool(os.environ.get("BASS_KERNEL_TRACE"))
    res = run_bass_kernel_spmd(
        nc, in_maps, core_ids=list(range(T_SH * F_SH)), trace=trace,
    )
    if trace:
        _NC_CACHE["last_exec_time_ns"] = res.exec_time_ns
        _NC_CACHE["last_results"] = res

    y = np.empty((N_TOK, O), dtype=np.float32)
    for core in range(T_SH * F_SH):
        ti, fi = core % T_SH, core // T_SH
        yb = res.results[core]["y"]                      # [nt, oq, p, j, n]
        yb = yb.transpose(0, 4, 1, 3, 2).reshape(N_SH, O_SH)
        y[ti * N_SH:(ti + 1) * N_SH, fi * O_SH:(fi + 1) * O_SH] = yb
    return y.reshape(B, S, O)


# revision 49
# speedup vs baseline: 1.0133x; 1.0133x over previous
"""Trainium2 Bass kernel for quantized-linear + LoRA (nn_LoRALinear).

Computes, for x:(4,2048,4096) f32, weight_quant:(4096,4096) i32 in [0,16),
scale/zero:(4096,1) f32, lora_A:(16,4096), lora_B:(4096,16), bias:(4096,):

    W = (weight_quant - zero) * scale
    y = x @ W.T + bias + 2.0 * (x @ lora_A.T) @ lora_B.T

Sharding across 8 NeuronCores: 4-way over tokens x 2-way over out-features.
Per core: x-slice (2048, 4096), weight rows slice (2048 of 4096), output
block (2048 tokens, 2048 features); host slices/permutes/dtype-repacks
inputs into partition-major blocks (so every DMA moves 4-16KB contiguous
per partition) and stitches/transposes output blocks.

Device algorithm (per core):

    P[o,n]   = sum_d (wq[o,d]-8) * x8[d,n]         (PE; all 32 d-chunks as
                                                    fp8e4 DoubleRow pairs)
             + sum_r B2[o,r] * t[r,n]              (K=17 fp32r matmul into the
             + (8-zero[o]) * rowsum[n]              same psum accumulation,
                                                    4x row-tiled)
    y[o,n]   = scale[o] * P[o,n] + bias[o]         (psum eviction)

with t = lora_A @ x.T (bf16, 4x column-tiled over d-chunks) augmented by a
ones-row giving rowsum, B2 = 2*lora_B/scale. The host ships x in bf16 and
fp8e4 (x8) and the centered weights (wq-8) in fp8e4 (exact for ints in
[-8,7]). The four column-tiled partial t groups are merged and replicated
across row-groups by one matmul against a host-built 0/1 matrix (DVE lanes
cannot cross partitions; the PE can). Output stays transposed per o-tile
group; the host de-transposes.
"""
import os
import sys
import types

sys.path.insert(0, "/opt/trn_rl_repo")

import numpy as np

import concourse.bass as bass
import concourse.mybir as mybir
import concourse.tile as tile
from concourse import bacc
from concourse.bass_utils import run_bass_kernel_spmd
from concourse.masks import make_identity

F32 = mybir.dt.float32
F32R = mybir.dt.float32r
BF16 = mybir.dt.bfloat16
FP8 = mybir.dt.float8e4
U8 = mybir.dt.uint8

DR = mybir.MatmulPerfMode.DoubleRow

NP_BF16 = mybir.dt.np(BF16)
NP_FP8 = mybir.dt.np(FP8)

# Problem shape (hardcoded per contract)
B, S, D, O, R = 4, 2048, 4096, 4096, 16
SCALING = 32.0 / 16.0
N_TOK = B * S            # 8192 tokens
T_SH, F_SH = 4, 2        # token shards x feature shards = 8 cores
N_SH = N_TOK // T_SH     # 2048 tokens per core
O_SH = O // F_SH         # 2048 out-features per core

NT = 4                   # n tiles per core
N_TILE = N_SH // NT      # 512
KC = D // 128            # 32 contraction chunks of 128
KQ = 4                   # k quarters (8 chunks each)
KD = KC // 2             # 16 DoubleRow chunk-pairs
OT = O_SH // 128         # 16 o tiles
OQ = 4                   # o tiles per psum pass (= o-column groups)
WQ_CENTER = 8.0          # weights shipped centered: wq - 8 (exact in fp8e4)


def _ensure_ntff_hook():
    """Best-effort: register the axon NTFF profile hook so trace=True works."""
    try:
        import antenv
        if "antenv.axon_hooks" not in sys.modules:
            hooks_mod = types.ModuleType("antenv.axon_hooks")
            hooks_mod._hook = None
            hooks_mod.set_axon_ntff_profile_hook = lambda h: setattr(hooks_mod, "_hook", h)
            hooks_mod.get_axon_ntff_profile_hook = lambda: hooks_mod._hook
            sys.modules["antenv.axon_hooks"] = hooks_mod
            antenv.axon_hooks = hooks_mod
        from trn_agent_boot.trn_boot import _ntff_profile_via_ctypes
        sys.modules["antenv.axon_hooks"].set_axon_ntff_profile_hook(
            _ntff_profile_via_ctypes("/opt/axon/libaxon_pjrt.so")
        )
        import concourse.bass_utils as bu
        bu.upload_artifacts = lambda tmpdir: tmpdir
    except Exception:
        pass


def build_nc() -> bass.Bass:
    nc = bacc.Bacc("TRN2", target_bir_lowering=False, debug=False)

    # host-pre-arranged partition-major layouts (big DMA packets):
    #  xt/xt8 [NT, KQ, 128, 8, N_TILE]   x.T blocks, bf16 / fp8
    #  wqt    [OQ, 128, KC, 512]         wq-8 per o-column group, fp8
    #  sbz    [128, 48]                  scale | bias | zero as [p, ot]
    #  y      [NT, OQ, 128, OQ, N_TILE]  transposed output blocks
    # x ships as two fp8 planes: x8 = fp8(bf16(x)) for the DR mains, and
    # xr8 = fp8(bf16(x) - x8), so lora/rowsum see x8+xr8 ~= bf16(x) while
    # moving half the bytes of a bf16 plane.
    xt_d = nc.dram_tensor("xt", (NT, KQ, 128, 8, N_TILE), FP8, kind="ExternalInput")
    xt8_d = nc.dram_tensor("xt8", (NT, KQ, 128, 8, N_TILE), FP8, kind="ExternalInput")
    wqt_d = nc.dram_tensor("wqt", (OQ, 128, KC, 512), FP8, kind="ExternalInput")
    sbz_d = nc.dram_tensor("sbz", (128, 3 * OT), F32, kind="ExternalInput")
    a_d = nc.dram_tensor("lora_at", (128, KC, R), BF16, kind="ExternalInput")
    b_d = nc.dram_tensor("lora_b", (128, OT * R), F32, kind="ExternalInput")
    # merge+replicate matrix: out[32j+r] = sum_i in[32i+r] for r<=16
    repm_d = nc.dram_tensor("repm", (128, 128), F32, kind="ExternalInput")
    y_d = nc.dram_tensor("y", (NT, OQ, 128, OQ, N_TILE), F32, kind="ExternalOutput")

    with tile.TileContext(nc) as tc:
        with (
            tc.tile_pool(name="const", bufs=1) as cpool,
            tc.tile_pool(name="wt", bufs=1) as wtpool,
            tc.tile_pool(name="xt", bufs=2) as xtpool,
            tc.tile_pool(name="xt8", bufs=2) as xt8pool,
            tc.tile_pool(name="stage", bufs=2) as stage,
            tc.tile_pool(name="tsb", bufs=2) as tsbpool,
            tc.tile_pool(name="outp", bufs=4) as outp,
            tc.tile_pool(name="ps_acc", bufs=7, space="PSUM") as ps_accp,
            tc.tile_pool(name="ps_t", bufs=1, space="PSUM") as ps_tp,
        ):
            # ---------------- PE warm-up: dependency-free matmuls so the HAM
            # clock-gate reaches K=8/8 before the real mains arrive.
            zeros_w = cpool.tile([128, N_TILE], BF16)
            nc.gpsimd.memset(zeros_w[:], 0.0)
            for _ in range(12):
                wps = ps_accp.tile([128, N_TILE], F32, tag="acc", name="wps")
                nc.tensor.matmul(
                    wps[:], zeros_w[:, 0:128], zeros_w[:],
                    start=True, stop=True,
                )

            # ---------------- constants (emitted onto the scalar queue
            # after nt0's first x-resid quarters; b2 needs them only by ~20us)
            sbz_sb = cpool.tile([128, 3 * OT], F32)
            scale_sb = sbz_sb[:, 0:OT]
            bias_sb = sbz_sb[:, OT:2 * OT]
            zero_sb = sbz_sb[:, 2 * OT:3 * OT]
            b_sb = cpool.tile([128, OT * R], F32)
            bblks = [b_sb[:, ot * R:(ot + 1) * R] for ot in range(OT)]
            repm_f = cpool.tile([128, 128], F32)

            def emit_const_loads():
                nc.scalar.dma_start(sbz_sb[:], sbz_d[:, :])
                nc.scalar.dma_start(b_sb[:], b_d[:, :])
                nc.scalar.dma_start(repm_f[:], repm_d[:, :])
                nc.vector.reciprocal(rcp_sb[:], scale_sb)
                nc.vector.tensor_scalar_mul(rcp2_sb[:], rcp_sb[:], float(SCALING))
                nc.vector.tensor_copy(repm[:], repm_f[:])

            ident_f = cpool.tile([128, 128], F32)
            make_identity(nc, ident_f)
            ident_r = cpool.tile([128, 128], F32R)
            nc.vector.tensor_copy(ident_r[:], ident_f[:])
            rcp_sb = cpool.tile([128, OT], F32)
            rcp2_sb = cpool.tile([128, OT], F32)
            repm = cpool.tile([128, 128], F32R)

            # A_augT [128, KC, 17] bf16: cols 0..15 = A.T chunk, col16 = ones.
            a4 = cpool.tile([128, KC, R + 1], BF16)
            nc.gpsimd.memset(a4[:, :, R:R + 1], 1.0)
            a_stg = cpool.tile([128, KC, R], BF16)

            def emit_a_load():
                nc.gpsimd.dma_start(a_stg[:], a_d[:, :, :])
                nc.vector.tensor_copy(a4[:, :, 0:R], a_stg[:])

            # b2r4 [128, OQ, 128] fp32r: for o-tile ot = oq*4+j, partition rows
            # 32j..32j+15 = (2*B/scale).T, row 32j+16 = (8 - zero) [pairs with
            # the rowsum row of t4]. Bias is folded into the psum eviction.
            # Built per o-group via one zero-padded [128,128] transpose so the
            # rows land at partition offsets 32j directly. Emitted after
            # mains(0,0) so it never blocks the head of the PE queue.
            b2r4 = cpool.tile([128, OQ, 128], F32R)

            def emit_b2_build():
                for oq in range(OQ):
                    pre32 = stage.tile([128, 128], F32R, tag="pre32")
                    nc.vector.tensor_copy(pre32[:], zeros_w[:, 0:128])
                    for j in range(4):
                        ot = oq * 4 + j
                        c = 32 * j
                        nc.vector.tensor_scalar(
                            out=pre32[:, c:c + R], in0=bblks[ot],
                            scalar1=rcp2_sb[:, ot:ot + 1], scalar2=None,
                            op0=mybir.AluOpType.mult,
                        )
                        nc.vector.tensor_scalar(
                            out=pre32[:, c + R:c + R + 1],
                            in0=zero_sb[:, ot:ot + 1],
                            scalar1=-1.0, scalar2=float(WQ_CENTER),
                            op0=mybir.AluOpType.mult, op1=mybir.AluOpType.add,
                        )
                    psb = ps_accp.tile([128, 128], F32R, tag="acc", name="psb")
                    nc.tensor.transpose(psb[:], pre32[:], ident_r[:])
                    nc.vector.tensor_copy(b2r4[:, oq, :], psb[:].bitcast(F32))

            # ---------------- weights: resident fp8, one tile per o-col group
            wt8s = [
                wtpool.tile([128, KC, 512], FP8, tag=f"wog{og}", name="wt8g")
                for og in range(OQ)
            ]

            def load_og(og, nsplit=1):
                kstep = KC // nsplit
                for s in range(nsplit):
                    k0, k1 = s * kstep, (s + 1) * kstep
                    nc.sync.dma_start(
                        wt8s[og][:, k0:k1, :], wqt_d[og, :, k0:k1, :]
                    )

            # ---------------- x loads (bf16 on scalar q, fp8 on gpsimd q) ----
            def alloc_x():
                xT = xtpool.tile([128, KC, N_TILE], FP8, tag="xT")
                xT8 = xt8pool.tile([128, KC, N_TILE], FP8, tag="xT8")
                return xT, xT8

            def load_x_quarter(nt, q, xT):
                nc.scalar.dma_start(
                    xT[:, 8 * q:8 * (q + 1), :], xt_d[nt, q, :, :, :]
                )

            def load_x8_quarter(nt, q, xT8, eng=None):
                (eng or nc.gpsimd).dma_start(
                    xT8[:, 8 * q:8 * (q + 1), :], xt8_d[nt, q, :, :, :]
                )

            def load_x_quarter2(nt, q, xT, eng):
                eng.dma_start(
                    xT[:, 8 * q:8 * (q + 1), :], xt_d[nt, q, :, :, :]
                )

            # ---------------- per-n-tile pieces ----------------
            def lora_proj(nt, xT, xT8):
                # t_aug: rows 0..15 = A@x.T, row16 = rowsum; 4x column-tiled
                # over d-chunks (chunk k -> col group k%4, output partitions
                # 32j..32j+16), then merged+replicated via repm on the PE.
                # Two accumulation passes: the x8 plane plus the residual.
                ps_t = ps_tp.tile([128, N_TILE], F32, tag="ps_t")
                for pi, xsrc in enumerate((xT8, xT)):
                    for t in range(KC // 4):
                        for j in range(4):
                            k = 4 * t + j
                            nc.tensor.matmul(
                                ps_t[32 * j:32 * j + R + 1, :],
                                a4[:, k, :], xsrc[:, k, :],
                                start=(pi == 0 and t == 0),
                                stop=(pi == 1 and t == KC // 4 - 1),
                                tile_position=(0, 32 * j),
                            )
                ts_sb = tsbpool.tile([128, N_TILE], F32R, tag="ts")
                if nt < 2:
                    # first use of each rotating buffer: zero it so the merge
                    # matmul sees no garbage in the never-written rows
                    nc.vector.tensor_copy(ts_sb[:], zeros_w[:])
                for j in range(4):
                    nc.vector.tensor_copy(
                        ts_sb[32 * j:32 * j + R + 1, :],
                        ps_t[32 * j:32 * j + R + 1, :],
                    )
                ps_t2 = ps_tp.tile([128, N_TILE], F32, tag="ps_t")
                nc.tensor.matmul(
                    ps_t2[:], repm[:], ts_sb[:],
                    start=True, stop=True,
                )
                t4 = tsbpool.tile([128, N_TILE], F32R, tag="t4")
                nc.vector.tensor_copy(t4[:], ps_t2[:])
                return t4

            def mains(oq, xT8):
                accs = []
                for _oi in range(OQ):
                    acc_tile = ps_accp.tile([128, N_TILE], F32, tag="acc")
                    accs.append(acc_tile)
                for kd in range(KD):
                    for oi in range(OQ):
                        nc.tensor.matmul(
                            accs[oi][:],
                            wt8s[oq][:, 2 * kd:2 * kd + 2,
                                     oi * 128:(oi + 1) * 128],
                            xT8[:, 2 * kd:2 * kd + 2, :],
                            start=(kd == 0), stop=False,
                            perf_mode=DR,
                        )
                return accs

            def tail(nt, oq, accs, t4):
                # lora + zero-correction: 4 concurrent row-tiled K=17 fp32r
                # matmuls closing the 4 accumulation banks.
                for j in range(OQ):
                    nc.tensor.matmul(
                        accs[j][:],
                        b2r4[32 * j:32 * j + R + 1, oq, :],
                        t4[32 * j:32 * j + R + 1, :],
                        start=False, stop=True,
                        tile_position=(32 * j, 0),
                    )
                yT4 = outp.tile([128, OQ, N_TILE], F32, tag="yT")
                for j in range(OQ):
                    ot = oq * OQ + j
                    # psum eviction y = scale[o]*P + bias[o]; last n-tile
                    # splits across engines so the final stores start sooner
                    if nt == NT - 1 and j % 2 == 1:
                        nc.vector.tensor_scalar(
                            out=yT4[:, j, :], in0=accs[j][:],
                            scalar1=scale_sb[:, ot:ot + 1],
                            scalar2=bias_sb[:, ot:ot + 1],
                            op0=mybir.AluOpType.mult,
                            op1=mybir.AluOpType.add,
                        )
                    else:
                        nc.scalar.activation(
                            yT4[:, j, :], accs[j][:],
                            mybir.ActivationFunctionType.Identity,
                            scale=scale_sb[:, ot:ot + 1],
                            bias=bias_sb[:, ot:ot + 1],
                        )
                # one batched store per o-group (8KB per partition); spread
                # late stores across both queues to cut the end tail.
                if nt >= NT - 2:
                    nc.sync.dma_start(y_d[nt, oq, :, 0:1, :], yT4[:, 0:1, :])
                    nc.scalar.dma_start(y_d[nt, oq, :, 1:2, :], yT4[:, 1:2, :])
                    nc.gpsimd.dma_start(y_d[nt, oq, :, 2:3, :], yT4[:, 2:3, :])
                    nc.sync.dma_start(y_d[nt, oq, :, 3:4, :], yT4[:, 3:4, :])
                else:
                    nc.sync.dma_start(y_d[nt, oq, :, :, :], yT4[:])

            # ---------------- schedule ----------------
            # startup: interleave og0 k-quarters with x8 halves on the fast
            # sync queue; bf16-x split between scalar and gpsimd queues.
            xT0, xT8_0 = alloc_x()
            # sync: og0-k0, x8-q0, rest of og0; gpsimd: x8-q1..q3 + a in
            # parallel -- the first mains need og0-k0 AND x8-q0 immediately.
            nc.sync.dma_start(wt8s[0][:, 0:8, :], wqt_d[0, :, 0:8, :])
            load_x8_quarter(0, 0, xT8_0, eng=nc.sync)
            load_x8_quarter(0, 1, xT8_0)
            nc.sync.dma_start(wt8s[0][:, 8:16, :], wqt_d[0, :, 8:16, :])
            load_x8_quarter(0, 2, xT8_0, eng=nc.sync)
            load_x8_quarter(0, 3, xT8_0)
            emit_a_load()
            nc.sync.dma_start(wt8s[0][:, 16:32, :], wqt_d[0, :, 16:32, :])
            load_x_quarter2(0, 0, xT0, nc.scalar)
            load_x_quarter2(0, 1, xT0, nc.gpsimd)
            emit_const_loads()
            load_x_quarter2(0, 2, xT0, nc.scalar)
            load_x_quarter2(0, 3, xT0, nc.gpsimd)

            xs = {0: (xT0, xT8_0)}
            for nt in range(NT):
                xT, xT8 = xs[nt]
                for oq in range(OQ):
                    accs = mains(oq, xT8)
                    if nt == 0 and oq == 0:
                        load_og(1)
                        emit_b2_build()
                    if oq == 0:
                        t4 = lora_proj(nt, xT, xT8)
                    tail(nt, oq, accs, t4)
                    if nt == 0 and oq == 0:
                        load_og(2)
                    if nt == 0 and oq == 1:
                        load_og(3)
                    # prefetch next n-tile's x, one quarter per oq pass
                    # (x8 on the fast sync queue; bf16 alternates scalar/gpsimd)
                    if nt + 1 < NT:
                        if oq == 0:
                            xs[nt + 1] = alloc_x()
                        nxT, nxT8 = xs[nt + 1]
                        load_x8_quarter(nt + 1, oq, nxT8, eng=nc.sync)
                        load_x_quarter2(nt + 1, oq, nxT,
                                        nc.scalar if oq % 2 == 0 else nc.gpsimd)

    nc.finalize()
    return nc


_NC_CACHE: dict = {}


def _get_nc() -> bass.Bass:
    if "nc" not in _NC_CACHE:
        _ensure_ntff_hook()
        _NC_CACHE["nc"] = build_nc()
    return _NC_CACHE["nc"]


def kernel(x, weight_quant, scale, zero, lora_A, lora_B, bias):
    x = np.ascontiguousarray(np.asarray(x, dtype=np.float32)).reshape(N_TOK, D)
    weight_quant = np.asarray(weight_quant, dtype=np.int32)
    scale_f = np.asarray(scale, dtype=np.float32).reshape(O)
    zero_f = np.asarray(zero, dtype=np.float32).reshape(O)
    bias_f = np.asarray(bias, dtype=np.float32).reshape(O)
    lora_A = np.asarray(lora_A, dtype=np.float32)
    lora_B = np.ascontiguousarray(np.asarray(lora_B, dtype=np.float32))

    nc = _get_nc()

    # host-side layout marshaling (slices/transposes/dtype repacks only)
    # [r, d] -> [128, KC, R]: a_t[p, k, r] = A[r, k*128 + p]
    lora_at = np.ascontiguousarray(
        lora_A.T.astype(NP_BF16).reshape(KC, 128, R).transpose(1, 0, 2))
    xt_by_t = []
    xt8_by_t = []
    for ti in range(T_SH):
        xsh = x[ti * N_SH:(ti + 1) * N_SH].astype(NP_BF16)          # [n, d]
        # [d, n] -> [q, kk, p, nt, n] -> [nt, q, p, kk, n]
        xb = xsh.T.reshape(KQ, 8, 128, NT, N_TILE).transpose(3, 0, 2, 1, 4)
        x8 = xb.astype(NP_FP8)
        xr = (xb.astype(np.float32)
              - x8.astype(np.float32)).astype(NP_FP8)                # residual
        xt_by_t.append(np.ascontiguousarray(xr))
        xt8_by_t.append(np.ascontiguousarray(x8))
    w8 = (weight_quant - 8).astype(np.float32).astype(NP_FP8)        # exact
    wqt_by_f = []
    for fi in range(F_SH):
        w8T = w8[fi * O_SH:(fi + 1) * O_SH].T                        # [d, o]
        # [d, o] -> [k, p, og, o] -> [og, p, k, o]
        wqt_by_f.append(np.ascontiguousarray(
            w8T.reshape(KC, 128, OQ, 512).transpose(2, 1, 0, 3)
        ))
    repm = np.zeros((128, 128), dtype=np.float32)
    for i in range(4):
        for j in range(4):
            for r in range(R + 1):
                repm[32 * i + r, 32 * j + r] = 1.0

    in_maps = []
    for core in range(T_SH * F_SH):
        ti, fi = core % T_SH, core // T_SH
        osl = slice(fi * O_SH, (fi + 1) * O_SH)
        sbz = np.concatenate([
            scale_f[osl].reshape(OT, 128).T,
            bias_f[osl].reshape(OT, 128).T,
            zero_f[osl].reshape(OT, 128).T,
        ], axis=1)                                                   # [128, 48]
        in_maps.append({
            "xt": xt_by_t[ti],
            "xt8": xt8_by_t[ti],
            "wqt": wqt_by_f[fi],
            "sbz": np.ascontiguousarray(sbz),
            "lora_at": lora_at,
            "lora_b": np.ascontiguousarray(
                lora_B[osl].reshape(OT, 128, R).transpose(1, 0, 2)
                .reshape(128, OT * R)),
            "repm": repm,
        })

    trace = bool(os.environ.get("BASS_KERNEL_TRACE"))
    res = run_bass_kernel_spmd(
        nc, in_maps, core_ids=list(range(T_SH * F_SH)), trace=trace,
    )
    if trace:
        _NC_CACHE["last_exec_time_ns"] = res.exec_time_ns
        _NC_CACHE["last_results"] = res

    y = np.empty((N_TOK, O), dtype=np.float32)
    for core in range(T_SH * F_SH):
        ti, fi = core % T_SH, core // T_SH
        yb = res.results[core]["y"]                      # [nt, oq, p, j, n]
        yb = yb.transpose(0, 4, 1, 3, 2).reshape(N_SH, O_SH)
        y[ti * N_SH:(ti + 1) * N_SH, fi * O_SH:(fi + 1) * O_SH] = yb
    return y.reshape(B, S, O)
